# revision 1
# baseline (speedup 1.0000x reference)
"""GQA causal attention block (B=4, S=1024, D=4096, H=32, KH=8, HD=128) on 8
Trainium2 NeuronCores.

Sharding: data-parallel over (batch, sequence-half) -> 8 independent cores, no
collectives. Each core computes the full attention output rows for its 512
query tokens (one half-sequence of one batch element), including Q/K/V
projections (K/V over the whole 1024-token sequence), RoPE, causal softmax
attention, and the output projection.

SPMD uniformity trick: the program is identical on every core; all per-core
variation (which tokens are queries, causality, RoPE angles) is carried in the
input DATA. Each core receives its batch's tokens permuted to [other-half,
own-half] order, so its query tokens always sit at positions [512:1024), and a
per-core additive mask column-permuted the same way encodes causality exactly.

RoPE trick: wq/wk columns are host-permuted within each head to [even dims,
odd dims] ("a|b" halves). Rotation then becomes rot(q) = q*cos + (S@q)*sin
with a constant 128x128 +-1 swap matrix S applied per head via one matmul
(dot products are invariant to the in-head permutation as long as q and k use
the same one; wv/wo are untouched).

Matmuls run in fp16 (same 11-bit mantissa as TF32/f32r, half the DMA bytes);
softmax statistics and normalization run in fp32. Scores are biased by -8
before exp (folded into the mask) so exp stays well inside fp16 range; the
softmax division removes the bias exactly.
"""

import numpy as np

import concourse.bass as bass
import concourse.tile as tile
from concourse import bacc, mybir
from concourse.bass_utils import run_bass_kernel_spmd

B, S, D = 4, 1024, 4096
H, KH, HD = 32, 8, 128
HALF = S // 2                    # tokens per core
N_CORES = 8
SCALE = 1.0 / float(np.sqrt(HD))
EXP_BIAS = -8.0                  # subtracted from scaled scores pre-exp
NEG = -1e9

MM_DT = mybir.dt.float16
MM_NP = np.float16
F32 = mybir.dt.float32
BF16 = mybir.dt.bfloat16

DT = D // 128                    # 32 d-tiles
QJT = H                          # 32 q-head j-tiles
KJT = KH                         # 8 kv j-tiles
TT = HALF // 128                 # 4 token-tiles per 512-chunk

_compiled = None


def _two_pi_split():
    two_pi = 2.0 * np.pi
    c1_bits = np.float32(two_pi).view(np.uint32) & np.uint32(0xFFFFF000)
    c1 = float(c1_bits.view(np.float32))        # ~12-bit mantissa head
    c2 = float(np.float64(two_pi) - np.float64(c1))
    return c1, c2


def _build():
    nc = bacc.Bacc("TRN2", target_bir_lowering=False, debug=False,
                   num_devices=N_CORES)

    x = nc.dram_tensor("x", [S, D], MM_DT, kind="ExternalInput").ap()
    wq = nc.dram_tensor("wq", [D, H * HD], MM_DT, kind="ExternalInput").ap()
    wk = nc.dram_tensor("wk", [D, KH * HD], MM_DT, kind="ExternalInput").ap()
    wv = nc.dram_tensor("wv", [D, KH * HD], MM_DT, kind="ExternalInput").ap()
    wo = nc.dram_tensor("wo", [H * HD, D], MM_DT, kind="ExternalInput").ap()
    maskT = nc.dram_tensor("maskT", [S, HALF], BF16, kind="ExternalInput").ap()
    freqsT = nc.dram_tensor("freqsT", [HD // 2, S], F32, kind="ExternalInput").ap()
    rotT_d = nc.dram_tensor("rotT", [128, 128], MM_DT, kind="ExternalInput").ap()
    ones_d = nc.dram_tensor("ones", [128, 128], MM_DT, kind="ExternalInput").ap()
    bias03_d = nc.dram_tensor("bias03", [128, 1], F32, kind="ExternalInput").ap()
    out = nc.dram_tensor("out", [HALF, D], F32, kind="ExternalOutput").ap()

    # DRAM spill buffers for projection outputs (re-streamed in attention)
    qT_d = nc.dram_tensor("qT_spill", [QJT, 128, HALF], MM_DT).ap()
    kT_d = nc.dram_tensor("kT_spill", [KJT, 128, S], MM_DT).ap()
    vT_d = nc.dram_tensor("vT_spill", [KJT, 128, S], MM_DT).ap()

    c1, c2 = _two_pi_split()
    INV_2PI = 1.0 / (2.0 * np.pi)
    PI_HALF = float(np.pi / 2)

    from contextlib import ExitStack

    es = ExitStack()
    with tile.TileContext(nc) as tc, es:
        const = es.enter_context(tc.tile_pool(name="const", bufs=1))
        trig = es.enter_context(tc.tile_pool(name="trig", bufs=1))
        trigw = es.enter_context(tc.tile_pool(name="trigw", bufs=3))
        maskp = es.enter_context(tc.tile_pool(name="maskp", bufs=1))
        big = es.enter_context(tc.tile_pool(name="big", bufs=2))
        kld = es.enter_context(tc.tile_pool(name="kld", bufs=3))
        wbuf = es.enter_context(tc.tile_pool(name="wbuf", bufs=4))
        pw = es.enter_context(tc.tile_pool(name="pw", bufs=2))
        sw = es.enter_context(tc.tile_pool(name="sw", bufs=3))
        pr = es.enter_context(tc.tile_pool(name="pr", bufs=4))
        qa = es.enter_context(tc.tile_pool(name="qa", bufs=2))
        vn = es.enter_context(tc.tile_pool(name="vn", bufs=2))
        rb = es.enter_context(tc.tile_pool(name="rb", bufs=2))
        ow = es.enter_context(tc.tile_pool(name="ow", bufs=4))
        ps_acc = es.enter_context(tc.tile_pool(name="ps_acc", bufs=5, space="PSUM"))
        ps_sc = es.enter_context(tc.tile_pool(name="ps_sc", bufs=3, space="PSUM"))

        # ---- constants ----
        rotT = const.tile([128, 128], MM_DT, tag="rot")
        nc.sync.dma_start(out=rotT, in_=rotT_d)
        ones = const.tile([128, 128], MM_DT, tag="ones")
        nc.sync.dma_start(out=ones, in_=ones_d)

        mask_t = maskp.tile([128, KJT, HALF], BF16, tag="mask")
        nc.sync.dma_start(
            out=mask_t, in_=maskT.rearrange("(t p) q -> p t q", p=128))

        # ---- cos/sin tables: [128, S]; rows 0:64 = freq f, 64:128 dup ----
        fr = trig.tile([64, S], F32, tag="fr")
        nc.sync.dma_start(out=fr, in_=freqsT)
        cosT = trig.tile([128, S], F32, tag="cos")
        sinT = trig.tile([128, S], F32, tag="sin")

        def trig_table(dst, shift, bias):
            t = trigw.tile([64, S], F32, tag="tw")
            nc.vector.tensor_scalar(out=t, in0=fr, scalar1=INV_2PI,
                                    scalar2=shift,
                                    op0=mybir.AluOpType.mult,
                                    op1=mybir.AluOpType.add)
            ni = trigw.tile([64, S], mybir.dt.int32, tag="tw")
            nc.vector.tensor_copy(ni, t)           # round-to-nearest
            nf = trigw.tile([64, S], F32, tag="tw")
            nc.vector.tensor_copy(nf, ni)
            r1 = trigw.tile([64, S], F32, tag="tw")
            nc.vector.scalar_tensor_tensor(
                out=r1, in0=nf, scalar=-c1, in1=fr,
                op0=mybir.AluOpType.mult, op1=mybir.AluOpType.add)
            r2 = trigw.tile([64, S], F32, tag="tw")
            nc.vector.scalar_tensor_tensor(
                out=r2, in0=nf, scalar=-c2, in1=r1,
                op0=mybir.AluOpType.mult, op1=mybir.AluOpType.add)
            if bias != 0.0:
                b_t = trigw.tile([64, 1], F32, tag="bias")
                nc.vector.memset(b_t, bias)
                nc.scalar.activation(dst[0:64, :], r2,
                                     mybir.ActivationFunctionType.Sin,
                                     bias=b_t)
            else:
                nc.scalar.activation(dst[0:64, :], r2,
                                     mybir.ActivationFunctionType.Sin)
            nc.sync.dma_start(out=dst[64:128, :], in_=dst[0:64, :])

        trig_table(sinT, 0.0, 0.0)
        trig_table(cosT, 0.25, PI_HALF)

        # ---- RoPE + evict helper ----
        def rope_evict(acc, cos_cols, sin_cols, dst):
            q_s = pw.tile([128, HALF], MM_DT, tag="qs")
            nc.scalar.copy(q_s, acc)
            ps2 = ps_sc.tile([128, HALF], F32, tag="sc")
            nc.tensor.matmul(ps2, rotT, q_s, start=True, stop=True)
            t1 = pw.tile([128, HALF], F32, tag="t1")
            nc.vector.tensor_mul(t1, q_s, cos_cols)
            t2 = pw.tile([128, HALF], F32, tag="t2")
            nc.vector.tensor_mul(t2, ps2, sin_cols)
            rot = pw.tile([128, HALF], MM_DT, tag="rotout")
            nc.vector.tensor_add(rot, t1, t2)
            nc.sync.dma_start(out=dst, in_=rot)

        def plain_evict(acc, dst):
            v_s = pw.tile([128, HALF], MM_DT, tag="qs")
            nc.scalar.copy(v_s, acc)
            nc.sync.dma_start(out=dst, in_=v_s)

        def transpose_x_chunk(c, xT):
            rows = slice(c * HALF, (c + 1) * HALF)
            for d in range(DT):
                nc.scalar.dma_start_transpose(
                    xT[:, d, :], x[rows, d * 128:(d + 1) * 128])

        def proj_group(w_ap, jg, xT):
            """4 j-tile outputs [128, HALF] accumulated over all of D."""
            accs = [ps_acc.tile([128, HALF], F32, tag="acc", name=f"acc{i}")
                    for i in range(4)]
            for d in range(DT):
                w_t = wbuf.tile([128, 512], MM_DT, tag="w")
                nc.sync.dma_start(
                    out=w_t, in_=w_ap[d * 128:(d + 1) * 128,
                                      jg * 512:(jg + 1) * 512])
                for jj in range(4):
                    nc.tensor.matmul(
                        accs[jj], w_t[:, jj * 128:(jj + 1) * 128],
                        xT[:, d, :], start=(d == 0), stop=(d == DT - 1))
            return accs

        # ---- phases T+P per token-chunk ----
        for c in range(2):
            tok = slice(c * HALF, (c + 1) * HALF)
            xT = big.tile([128, DT, HALF], MM_DT, tag="big")
            transpose_x_chunk(c, xT)

            for w_ap, spill, do_rope in ((wk, kT_d, True), (wv, vT_d, False)):
                for ug in range(KJT // 4):
                    accs = proj_group(w_ap, ug, xT)
                    for jj in range(4):
                        j = ug * 4 + jj
                        if do_rope:
                            rope_evict(accs[jj], cosT[:, tok], sinT[:, tok],
                                       spill[j, :, tok])
                        else:
                            plain_evict(accs[jj], spill[j, :, tok])

            if c == 1:
                for jg in range(QJT // 4):
                    accs = proj_group(wq, jg, xT)
                    for jj in range(4):
                        rope_evict(accs[jj], cosT[:, HALF:], sinT[:, HALF:],
                                   qT_d[jg * 4 + jj])

        # ---- attention ----
        # per-tile exp bias vectors (per-partition scalars for ACT):
        # bias03: per-core input (-8 if other-half keys are past, -1e9 if
        # future); m8: constant -8 (unmasked own-half region).
        bias03_t = const.tile([128, 1], F32, tag="b03")
        nc.sync.dma_start(out=bias03_t, in_=bias03_d)
        m8_t = const.tile([128, 1], F32, tag="m8")
        nc.vector.memset(m8_t, EXP_BIAS)
        neg_t = const.tile([128, 1], F32, tag="negb")
        nc.vector.memset(neg_t, NEG)

        def attend_head(h, kT_s, v_n, attnT):
            q_s = qa.tile([128, HALF], MM_DT, tag="qa")
            nc.sync.dma_start(out=q_s, in_=qT_d[h])
            oT_ps = ps_acc.tile([128, HALF], F32, tag="acc")
            sum_ps = ps_acc.tile([128, HALF], F32, tag="acc")
            ntau = S // 128
            for tau in range(ntau):
                sc_ps = ps_sc.tile([128, HALF], F32, tag="sc")
                nc.tensor.matmul(
                    sc_ps, kT_s[:, tau * 128:(tau + 1) * 128], q_s,
                    start=True, stop=True)
                p_t = pr.tile([128, HALF], MM_DT, tag="pr")
                if tau < 4:
                    # other-half keys: mask is uniform per core -> ACT bias
                    nc.scalar.activation(p_t, sc_ps,
                                         mybir.ActivationFunctionType.Exp,
                                         bias=bias03_t, scale=SCALE)
                else:
                    # own-half keys: causal mask tile (includes -8 bias)
                    sc_s = sw.tile([128, HALF], F32, tag="sw")
                    nc.vector.scalar_tensor_tensor(
                        out=sc_s, in0=sc_ps, scalar=SCALE,
                        in1=mask_t[:, tau, :],
                        op0=mybir.AluOpType.mult, op1=mybir.AluOpType.add)
                    nc.scalar.activation(p_t, sc_s,
                                         mybir.ActivationFunctionType.Exp)
                nc.tensor.matmul(oT_ps, v_n[:, tau, :], p_t, start=(tau == 0),
                                 stop=(tau == ntau - 1))
                nc.tensor.matmul(sum_ps, ones, p_t, start=(tau == 0),
                                 stop=(tau == ntau - 1))
            rB2 = rb.tile([128, HALF], F32, tag="rb2")
            nc.vector.reciprocal_approx_fast(rB2, sum_ps)
            nc.vector.tensor_mul(attnT[:, h, :], oT_ps, rB2)

        attnT = big.tile([128, H, HALF], MM_DT, tag="big")
        for kh in range(KH):
            kT_s = kld.tile([128, S], MM_DT, tag="kld")
            nc.sync.dma_start(out=kT_s, in_=kT_d[kh])
            v_n = vn.tile([128, KJT, 128], MM_DT, tag="vn")
            for tau in range(S // 128):
                nc.scalar.dma_start_transpose(
                    v_n[:, tau, :], vT_d[kh, :, tau * 128:(tau + 1) * 128])
            for qi in range(H // KH):
                attend_head(kh * (H // KH) + qi, kT_s, v_n, attnT)

        # ---- output projection ----
        def out_group(djg, attnT):
            accs = [ps_acc.tile([128, 512], F32, tag="acc", name=f"oacc{i}")
                    for i in range(TT)]
            for hd in range(H):
                w_t = wbuf.tile([128, 512], MM_DT, tag="w")
                nc.sync.dma_start(
                    out=w_t, in_=wo[hd * 128:(hd + 1) * 128,
                                    djg * 512:(djg + 1) * 512])
                for t4 in range(TT):
                    nc.tensor.matmul(
                        accs[t4], attnT[:, hd, t4 * 128:(t4 + 1) * 128],
                        w_t, start=(hd == 0), stop=(hd == H - 1))
            for t4 in range(TT):
                o_s = ow.tile([128, 512], F32, tag="ow")
                nc.any.tensor_copy(o_s, accs[t4])
                nc.sync.dma_start(
                    out=out[t4 * 128:(t4 + 1) * 128,
                            djg * 512:(djg + 1) * 512],
                    in_=o_s)

        for djg in range(D // 512):
            out_group(djg, attnT)

    nc.compile()
    return nc


def _get_compiled():
    global _compiled
    if _compiled is None:
        _compiled = _build()
    return _compiled


def _host_prep(x, freqs_cis, mask, wq, wk, wv, wo):
    """Shard + lay out inputs per core. Core c: batch c//2, seq-half c%2."""
    # in-head column permutation: [even dims, odd dims] per head
    def ab_perm(n_heads):
        p = []
        for h in range(n_heads):
            base = h * HD
            p.extend(range(base, base + HD, 2))
            p.extend(range(base + 1, base + HD, 2))
        return np.asarray(p)

    wq_p = np.ascontiguousarray(wq[:, ab_perm(H)]).astype(MM_NP)
    wk_p = np.ascontiguousarray(wk[:, ab_perm(KH)]).astype(MM_NP)
    wv_p = wv.astype(MM_NP)
    wo_p = wo.astype(MM_NP)

    # S^T for rot(q) = q*cos + (S@q)*sin with [a|b] layout:
    # S = [[0, -I],[I, 0]] (ra = -b rows, rb = a rows) -> S^T = [[0, I],[-I, 0]]
    rotT = np.zeros((128, 128), dtype=MM_NP)
    rotT[np.arange(64), np.arange(64) + 64] = 1.0
    rotT[np.arange(64) + 64, np.arange(64)] = -1.0
    ones = np.ones((128, 128), dtype=MM_NP)

    mask = np.asarray(mask, dtype=np.float32)
    freqs = np.asarray(freqs_cis, dtype=np.float32)
    import ml_dtypes

    in_maps = []
    for c in range(N_CORES):
        b, hhalf = divmod(c, 2)
        own = np.arange(hhalf * HALF, (hhalf + 1) * HALF)
        other = np.arange((1 - hhalf) * HALF, (2 - hhalf) * HALF)
        perm = np.concatenate([other, own])
        x_c = np.ascontiguousarray(x[b][perm]).astype(MM_NP)
        freqsT_c = np.ascontiguousarray(freqs[perm].T)          # [64, S]
        m = mask[own][:, perm] + np.float32(EXP_BIAS)           # [512, S]
        maskT_c = np.ascontiguousarray(m.T).astype(ml_dtypes.bfloat16)
        bias03 = np.full((128, 1),
                         NEG if hhalf == 0 else EXP_BIAS, dtype=np.float32)
        in_maps.append({
            "x": x_c, "wq": wq_p, "wk": wk_p, "wv": wv_p, "wo": wo_p,
            "maskT": maskT_c, "freqsT": freqsT_c,
            "rotT": rotT, "ones": ones, "bias03": bias03,
        })
    return in_maps


def kernel(x, freqs_cis, mask, wq, wk, wv, wo):
    nc = _get_compiled()
    in_maps = _host_prep(x, freqs_cis, mask, wq, wk, wv, wo)
    res = run_bass_kernel_spmd(nc, in_maps, list(range(N_CORES)))
    out = np.empty((B, S, D), dtype=np.float32)
    for c in range(N_CORES):
        b, hhalf = divmod(c, 2)
        out[b, hhalf * HALF:(hhalf + 1) * HALF, :] = res.results[c]["out"]
    return out

